# revision 21
# baseline (speedup 1.0000x reference)
"""DANetHead (position attention + channel attention + conv/BN/ReLU) on 8
Trainium2 NeuronCores via Bass/Tile.

Sharding: data-parallel over batch (4) x image-row-halves (2) = 8 cores.
Each core computes a 34-row window (32 own rows + 1 halo row each side) of
one batch item.  The window is position-uniform across cores (host-side roll
of the pixel axis), so one SPMD program serves all 8 cores; per-core
behaviour differs only through input data.

v2 design (vs v1):
  - attention output accumulated DIRECTLY as pa[c, m] (lhsT = v^T chunks,
    rhs = exp(e^T)) -- no PE transposes at all.
  - energy chunk-PAIRS row-packed into the 128x128 array (K=64 each half:
    even chunks on rows 0:64, odd on rows 64:128 via a partition-duplicated
    q^T and an even/odd-packed k), one Exp activation per [128, 2*bsz] pair.
  - softmax denominator via an all-ones fp32 matmul contracting the
    DVE-accumulated esum over partitions -- result lands pre-broadcast in
    all 128 partitions; reciprocal + per-column multiply on DVE.
  - residual (+bv) provided host-side in [c, m] layout (xball), fused into
    the ca write together with max/sum pooling (tensor_tensor_reduce).
  - pooled-stats pair exchange via remote_dma_broadcast (peer SBUF write,
    ~3us) instead of a CC-firmware AllGather (~27us), emitted mid-halo-block.
  - all weights host-pre-rearranged to SBUF layouts (single-descriptor DMA),
    weights-first + fine-sliced xf load order so the PE starts early.
  - conv weights SE-scaled in place, oc=0 half first so conv starts early.
"""

import numpy as np
import ml_dtypes

import os
import concourse.bass as bass
import concourse.mybir as mybir
import concourse.tile as tile

SKIP_XCHG = os.environ.get("SKIP_XCHG") == "1"
NOPACK = os.environ.get("PACK") != "1"
BF16_DEN = os.environ.get("BF16_DEN") == "1"
QDUP_PE = os.environ.get("QDUP_PE") == "1"

BF16 = ml_dtypes.bfloat16
F32 = np.float32

P = 128
CIN = 512            # channels
NPIX = 4096          # 64*64 pixels
C8 = 64              # q/k channels
OC = 256             # conv output channels
M = 2176             # per-core pixel window: 34 rows * 64
OWN = 2048           # own pixels (window rows 1..32, first in m order)
NCH = 32             # 128-pixel n-chunks
NPAIR = 16           # chunk pairs
BLOCKS = [(0, 512), (512, 512), (1024, 512), (1536, 512), (2048, 128)]

BN_EPS = 1e-5

_BUILD_CACHE = {}


def _emit(tc, nc, t):
    fp32 = mybir.dt.float32
    bf16 = mybir.dt.bfloat16
    Act = mybir.ActivationFunctionType
    Alu = mybir.AluOpType
    XY = mybir.AxisListType.XY

    import contextlib
    ctx = contextlib.ExitStack()

    persist = ctx.enter_context(tc.tile_pool(name="persist", bufs=1))
    vt_pool = ctx.enter_context(tc.tile_pool(name="vt", bufs=NCH))
    xf_pool = ctx.enter_context(tc.tile_pool(name="xf", bufs=4))
    work = ctx.enter_context(tc.tile_pool(name="work", bufs=2))
    out_pool = ctx.enter_context(tc.tile_pool(name="yout", bufs=3))
    small = ctx.enter_context(tc.tile_pool(name="small", bufs=2))

    ps_pair = ctx.enter_context(tc.tile_pool(name="ps_pair", bufs=1, space="PSUM"))
    ps_pa = ctx.enter_context(tc.tile_pool(name="ps_pa", bufs=5, space="PSUM"))
    ps_den = ctx.enter_context(tc.tile_pool(name="ps_den", bufs=1, space="PSUM"))

    # ---------------- loads (weights first, xf in fine slices) ----------
    wq_sb = persist.tile([P, 4, C8], bf16)
    nc.sync.dma_start(out=wq_sb, in_=t["wq_p"][:, :, :])
    bq_sb = persist.tile([C8, 1], fp32)
    nc.sync.dma_start(out=bq_sb, in_=t["bq"][:, :])

    xf_sb = [xf_pool.tile([P, NPIX], bf16, tag="xf", name=f"xf{ci}")
             for ci in range(4)]
    xf_groups = [(0, 512), (512, 512), (1024, 1024), (2048, 1024), (3072, 1024)]
    for s, (goff, gw) in enumerate(xf_groups):
        sl = slice(goff, goff + gw)
        if s == 2:
            wk_sb = persist.tile([P, 4, C8], bf16)
            nc.sync.dma_start(out=wk_sb, in_=t["wk_p"][:, :, :])
            bk2_sb = persist.tile([P, 1], fp32)
            nc.sync.dma_start(out=bk2_sb, in_=t["bk2"][:, :])
        if s == 3:
            wv_sb = persist.tile([P, 4, CIN], bf16)
            nc.sync.dma_start(out=wv_sb, in_=t["wv_p"][:, :, :])
        for ci in range(4):
            nc.sync.dma_start(out=xf_sb[ci][:, sl], in_=t["xf"][ci * P:(ci + 1) * P, sl])

    # late loads
    xball_sb = persist.tile([P, 4, M], bf16)
    nc.sync.dma_start(out=xball_sb, in_=t["xball"][:, :, :])
    pmask_sb = persist.tile([P, P], fp32)
    nc.sync.dma_start(out=pmask_sb, in_=t["pmask_bc"][:, :])
    w1_sb = persist.tile([P, 4, C8], bf16)
    nc.sync.dma_start(out=w1_sb, in_=t["w1_p"][:, :, :])
    w2_sb = persist.tile([C8, 4, P], bf16)
    nc.sync.dma_start(out=w2_sb, in_=t["w2_p"][:, :, :])
    cw_sb = persist.tile([P, 36, OC], bf16)
    nc.sync.dma_start(out=cw_sb, in_=t["cw_p"][:, :, :])
    bns_sb = persist.tile([P, 2], fp32)
    nc.sync.dma_start(out=bns_sb, in_=t["bns"][:, :])
    bnb_sb = persist.tile([P, 2], fp32)
    nc.sync.dma_start(out=bnb_sb, in_=t["bnb"][:, :])

    # Dummy 8-core AllGather: its only purpose is to make the NEFF a CC
    # participant so the runtime co-schedules all 8 cores (otherwise cores
    # launch ms apart and the P2P stats exchange eats the skew).  Runs on
    # the CC cores during the load phase; nothing reads its output.
    rsem = nc.alloc_semaphore("xchg_r")
    lsem = nc.alloc_semaphore("xchg_l")
    psem = nc.alloc_semaphore("xchg_p")
    dram = ctx.enter_context(tc.tile_pool(name="dram", bufs=1, space="DRAM"))
    bar_sb = small.tile([P, 1], fp32, tag="bar", bufs=1)
    nc.vector.memset(bar_sb, 0.0)
    bar_in = dram.tile([P, 1], fp32, tag="bar_in")
    bar_out = dram.tile([8, P, 1], fp32, tag="bar_out")
    nc.sync.dma_start(out=bar_in, in_=bar_sb)
    nc.gpsimd.collective_compute(
        "AllGather", Alu.bypass,
        replica_groups=[[0, 1, 2, 3, 4, 5, 6, 7]],
        ins=[bar_in.opt()], outs=[bar_out.opt()])
    bar2_sb = small.tile([P, 8], fp32, tag="bar2", bufs=1)
    nc.sync.dma_start(out=bar2_sb,
                      in_=bar_out.rearrange("r p one -> p (r one)"))
    with tc.tile_critical(name="align"):
        nc.vector.tensor_scalar_mul(bar2_sb, bar2_sb, 0.0)
        nc.gpsimd.sem_clear(rsem)
        nc.gpsimd.sem_clear(lsem)
        nc.gpsimd.sem_clear(psem)

    ones_sb = persist.tile([P, P], fp32)
    nc.vector.memset(ones_sb, 1.0)
    ones16_sb = persist.tile([P, P], bf16)
    nc.vector.memset(ones16_sb, 1.0)

    # ca: [c-part, cc, 34 rows, 66 cols], zero column pads
    ca_sb = persist.tile([P, 4, 34 * 66], bf16)
    cav4 = ca_sb.rearrange("p c (r x) -> p c r x", x=66)
    for cc in range(4):
        nc.vector.memset(cav4[:, cc, :, 0:1], 0.0)
        nc.vector.memset(cav4[:, cc, :, 65:66], 0.0)

    # ---------------- q projection -> qT2 (duplicated partitions) -------
    qT2 = persist.tile([P, M], bf16)
    for b in range(4):
        q_ps = ps_den.tile([C8, 512], fp32, tag="den")
        for ci in range(4):
            nc.tensor.matmul(q_ps, lhsT=wq_sb[:, ci, :],
                             rhs=xf_sb[ci][:, 64 + 512 * b:576 + 512 * b],
                             start=(ci == 0), stop=(ci == 3))
        nc.scalar.activation(qT2[0:C8, 512 * b:512 * (b + 1)], q_ps,
                             Act.Identity, bias=bq_sb[:, 0:1])
    qh_ps = ps_den.tile([C8, P], fp32, tag="den")
    for ci in range(4):
        nc.tensor.matmul(qh_ps[:, 0:64], lhsT=wq_sb[:, ci, :],
                         rhs=xf_sb[ci][:, 0:64], start=(ci == 0), stop=(ci == 3))
    for ci in range(4):
        nc.tensor.matmul(qh_ps[:, 64:128], lhsT=wq_sb[:, ci, :],
                         rhs=xf_sb[ci][:, OWN + 64:OWN + 128],
                         start=(ci == 0), stop=(ci == 3))
    nc.scalar.activation(qT2[0:C8, OWN:OWN + P], qh_ps,
                         Act.Identity, bias=bq_sb[:, 0:1])
    # duplicate q^T to partitions 64:128
    if QDUP_PE:
        for b in range(4):
            q_ps2 = ps_den.tile([C8, 512], fp32, tag="den", name=f"qps2_{b}")
            for ci in range(4):
                nc.tensor.matmul(q_ps2, lhsT=wq_sb[:, ci, :],
                                 rhs=xf_sb[ci][:, 64 + 512 * b:576 + 512 * b],
                                 start=(ci == 0), stop=(ci == 3))
            nc.scalar.activation(qT2[C8:P, 512 * b:512 * (b + 1)], q_ps2,
                                 Act.Identity, bias=bq_sb[:, 0:1])
        qh_ps2 = ps_den.tile([C8, P], fp32, tag="den")
        for ci in range(4):
            nc.tensor.matmul(qh_ps2[:, 0:64], lhsT=wq_sb[:, ci, :],
                             rhs=xf_sb[ci][:, 0:64], start=(ci == 0), stop=(ci == 3))
        for ci in range(4):
            nc.tensor.matmul(qh_ps2[:, 64:128], lhsT=wq_sb[:, ci, :],
                             rhs=xf_sb[ci][:, OWN + 64:OWN + 128],
                             start=(ci == 0), stop=(ci == 3))
        nc.scalar.activation(qT2[C8:P, OWN:OWN + P], qh_ps2,
                             Act.Identity, bias=bq_sb[:, 0:1])
    else:
        nc.sync.dma_start(out=qT2[C8:P, :], in_=qT2[0:C8, :])

    # ---------------- k projection, even/odd packed ----------------------
    # kpack[0:64, p0, :] = k for chunk 2*p0 ; kpack[64:128, p0, :] = 2*p0+1
    if NOPACK:
        k_sb = persist.tile([C8, NPIX], bf16)
        for j in range(8):
            k_ps = ps_den.tile([C8, 512], fp32, tag="den", name=f"kps{j}")
            for ci in range(4):
                nc.tensor.matmul(k_ps, lhsT=wk_sb[:, ci, :],
                                 rhs=xf_sb[ci][:, 512 * j:512 * (j + 1)],
                                 start=(ci == 0), stop=(ci == 3))
            nc.scalar.activation(k_sb[:, 512 * j:512 * (j + 1)], k_ps,
                                 Act.Identity, bias=bq_sb[:, 0:1] if False else bk2_sb[0:C8, 0:1])
        kpack = None
    else:
        kpack = persist.tile([P, NPAIR, P], bf16)
        xv = [xf_sb[ci].rearrange("p (a e b) -> p a e b", e=2, b=P)
              for ci in range(4)]   # a = pair index, e = even/odd within pair
        for j in range(8):          # 512-pixel column blocks (pairs 2j, 2j+1)
            kp_ps = ps_den.tile([P, 2, P], fp32, tag="den", name=f"kpps{j}")
            for ci in range(4):     # even chunks -> partitions 0:64
                nc.tensor.matmul(kp_ps[0:C8, :, :], lhsT=wk_sb[:, ci, :],
                                 rhs=xv[ci][:, 2 * j:2 * j + 2, 0, :],
                                 start=(ci == 0), stop=(ci == 3))
            for ci in range(4):     # odd chunks -> partitions 64:128
                nc.tensor.matmul(kp_ps[C8:P, :, :], lhsT=wk_sb[:, ci, :],
                                 rhs=xv[ci][:, 2 * j:2 * j + 2, 1, :],
                                 start=(ci == 0), stop=(ci == 3))
            nc.scalar.activation(kpack[:, 2 * j:2 * j + 2, :], kp_ps,
                                 Act.Identity, bias=bk2_sb[:, 0:1])

    # ---------------- v^T ------------------------------------------------
    vt_sb = []
    for nch in range(NCH):
        v_ps = ps_pa.tile([P, CIN], fp32, tag="pa")
        for ci in range(4):
            nc.tensor.matmul(v_ps, lhsT=xf_sb[ci][:, nch * P:(nch + 1) * P],
                             rhs=wv_sb[:, ci, :], start=(ci == 0), stop=(ci == 3))
        vt = vt_pool.tile([P, CIN], bf16, tag="vt")
        nc.vector.tensor_copy(vt, v_ps)
        vt_sb.append(vt)

    # ---------------- attention -----------------------------------------
    pool_s = small.tile([P, 4], fp32, tag="pool_s", bufs=1)
    pool_m = small.tile([P, 4], fp32, tag="pool_m", bufs=1)
    pool_sm = persist.tile([P, 2 * 4], fp32)     # [ (2 stats, 4 cc) ]
    pool_rm = persist.tile([P, 2 * 4], fp32)     # partner's, remote-written

    def emit_pair(bi, boff, bsz, p0, state):
        """energy pair + exp + esum; 1-lag pa of previous pair."""
        pair_ps = ps_pair.tile([P, 2, bsz], fp32, tag="pair", name=f"pr{bi}_{p0}")
        if NOPACK:
            nc.tensor.matmul(pair_ps[:, 0, :],
                             lhsT=k_sb[:, (2 * p0) * P:(2 * p0 + 1) * P],
                             rhs=qT2[0:C8, boff:boff + bsz], start=True, stop=True)
            nc.tensor.matmul(pair_ps[:, 1, :],
                             lhsT=k_sb[:, (2 * p0 + 1) * P:(2 * p0 + 2) * P],
                             rhs=qT2[0:C8, boff:boff + bsz], start=True, stop=True)
        else:
            nc.tensor.matmul(pair_ps[:, 0, :], lhsT=kpack[0:C8, p0, :],
                             rhs=qT2[0:C8, boff:boff + bsz], start=True, stop=True)
            nc.tensor.matmul(pair_ps[:, 1, :], lhsT=kpack[C8:P, p0, :],
                             rhs=qT2[C8:P, boff:boff + bsz], start=True, stop=True)
        expt = work.tile([P, 2, bsz], bf16, tag="expt", bufs=3)
        nc.scalar.activation(expt, pair_ps, Act.Exp)
        if p0 == 0:
            nc.vector.tensor_copy(state["esum"], expt)
        else:
            nc.vector.tensor_add(state["esum"], state["esum"], expt)
        state["pending"].append((p0, expt))

    def emit_pa(bi, p0, expt, state):
        for h in range(2):
            nch = 2 * p0 + h
            for cc in range(4):
                nc.tensor.matmul(state["pa"][cc],
                                 lhsT=vt_sb[nch][:, cc * P:(cc + 1) * P],
                                 rhs=expt[:, h, :],
                                 start=(nch == 0), stop=(nch == NCH - 1))

    def tail_a(bi, boff, bsz, state):
        """den (pre-broadcast), recip, pnorm -- the pa-bank readers."""
        den_ps = ps_den.tile([P, bsz], fp32, tag="den", name=f"den{bi}")
        if BF16_DEN:
            esb = work.tile([P, 2, bsz], bf16, tag="esb", bufs=2,
                            name=f"esb{bi}")
            nc.vector.tensor_copy(esb, state["esum"])
            nc.tensor.matmul(den_ps, lhsT=ones16_sb, rhs=esb[:, 0, :],
                             start=True, stop=False)
            nc.tensor.matmul(den_ps, lhsT=ones16_sb, rhs=esb[:, 1, :],
                             start=False, stop=True)
        else:
            nc.tensor.matmul(den_ps, lhsT=ones_sb, rhs=state["esum"][:, 0, :],
                             start=True, stop=False)
            nc.tensor.matmul(den_ps, lhsT=ones_sb, rhs=state["esum"][:, 1, :],
                             start=False, stop=True)
        recip = work.tile([P, bsz], fp32, tag="recip", bufs=2,
                          name=f"recip{bi}")
        nc.vector.reciprocal_approx_fast(out=recip, in_=den_ps)
        if bi == 4:
            nc.vector.tensor_mul(recip, recip, pmask_sb)
        state["pnorm"] = []
        for cc in range(4):
            pnorm = work.tile([P, bsz], fp32, tag="pnorm", bufs=4,
                              name=f"pn{bi}_{cc}")
            nc.vector.tensor_mul(pnorm, state["pa"][cc], recip)
            state["pnorm"].append(pnorm)

    def tail_b(bi, boff, bsz, state):
        """normalized + residual -> ca (+ pooling for own blocks)."""
        r0 = 1 + 8 * bi
        for cc in range(4):
            pnorm = state["pnorm"][cc]
            xb = xball_sb[:, cc, boff:boff + bsz]
            if bi < 4:
                cav = cav4[:, cc, r0:r0 + 8, 1:65]
                pn3 = pnorm.rearrange("p (r x) -> p r x", x=64)
                xb3 = xb.rearrange("p (r x) -> p r x", x=64)
                ptm = small.tile([P, 4, 2], fp32, tag="ptm")
                nc.vector.tensor_add(cav, pn3, xb3)
                nc.vector.reduce_max(ptm[:, cc, 0:1], cav, axis=XY)
                nc.vector.reduce_sum(ptm[:, cc, 1:2], cav, axis=XY)
                if bi == 0:
                    nc.vector.tensor_copy(pool_m[:, cc:cc + 1], ptm[:, cc, 0:1])
                    nc.vector.tensor_copy(pool_s[:, cc:cc + 1], ptm[:, cc, 1:2])
                else:
                    nc.vector.tensor_max(pool_m[:, cc:cc + 1],
                                         pool_m[:, cc:cc + 1], ptm[:, cc, 0:1])
                    nc.vector.tensor_add(pool_s[:, cc:cc + 1],
                                         pool_s[:, cc:cc + 1], ptm[:, cc, 1:2])
            else:
                nc.vector.tensor_add(cav4[:, cc, 0:1, 1:65],
                                     pnorm[:, 0:64], xb[:, 0:64])
                nc.vector.tensor_add(cav4[:, cc, 33:34, 1:65],
                                     pnorm[:, 64:128], xb[:, 64:128])

    # ---- attention blocks, software-pipelined tails ----
    # block bi's tail (den/recip/normalize/ca) is emitted 3 pairs into block
    # bi+1 so the PE never waits on the DVE tail chain.  The SE-stats
    # exchange + MLP are interleaved with the halo block's pairs.
    psm = pool_sm.rearrange("p (s c) -> p s c", c=4)
    prm = pool_rm.rearrange("p (s c) -> p s c", c=4)
    zs_sb = small.tile([P, 4], fp32, tag="zs")
    zm_sb = small.tile([P, 4], fp32, tag="zm")
    rhs_z = small.tile([P, 4, 2], bf16, tag="rhs_z")
    h_sb = small.tile([C8, 2], bf16, tag="h_sb")
    stot = small.tile([P, 4], fp32, tag="stot")
    s_sb = small.tile([P, 4, 2], fp32, tag="s_sb", bufs=1)
    es = small.tile([P, 4], fp32, tag="es")
    scale_sb = small.tile([P, 4], fp32, tag="scale")
    cwv = cw_sb.rearrange("p (t c) o -> p c t o", c=4)

    def emit_send():
        if SKIP_XCHG:
            return
        with tc.tile_critical(name="xchg_send", no_gpsimd_drain=True):
            nc.gpsimd.remote_dma_broadcast(
                out_ap=pool_rm[:, :], in_ap=pool_sm[:, :],
                remote_sem=rsem, local_sem=lsem,
                rdests=[(0, 1), None, None, None, None, None, None, None],
            ).then_inc(psem, 1)
            nc.gpsimd.wait_ge(psem, 1)
            nc.gpsimd.trigger_dma(count=1)

    def emit_wait_combine():
        if SKIP_XCHG:
            nc.vector.tensor_copy(pool_rm[:, :], pool_sm[:, :])
        else:
            with tc.tile_critical(name="xchg_wait"):
                nc.vector.wait_ge(rsem, 2)
        nc.vector.tensor_add(zs_sb, psm[:, 0, :], prm[:, 0, :])
        nc.vector.tensor_max(zm_sb, psm[:, 1, :], prm[:, 1, :])
        nc.vector.tensor_scalar_mul(rhs_z[:, :, 0], zs_sb, 1.0 / float(NPIX))
        nc.vector.tensor_copy(rhs_z[:, :, 1], zm_sb)

    def emit_mlp():
        h_ps = ps_den.tile([C8, 2], fp32, tag="den")
        for cc in range(4):
            nc.tensor.matmul(h_ps, lhsT=w1_sb[:, cc, :], rhs=rhs_z[:, cc, :],
                             start=(cc == 0), stop=(cc == 3))
        nc.vector.tensor_scalar_max(h_sb, h_ps, 0.0)
        for cc in range(4):
            s_ps = ps_den.tile([P, 2], fp32, tag="den", name=f"s_ps{cc}")
            nc.tensor.matmul(s_ps, lhsT=w2_sb[:, cc, :], rhs=h_sb,
                             start=True, stop=True)
            nc.vector.tensor_copy(s_sb[:, cc, :], s_ps)
            nc.vector.tensor_add(stot[:, cc:cc + 1], s_sb[:, cc, 0:1],
                                 s_sb[:, cc, 1:2])

    def emit_sigmoid_scale(oc):
        if oc == 0:
            nc.scalar.activation(es, stot, Act.Exp, scale=-1.0)
            nc.vector.tensor_scalar_add(es, es, 1.0)
            nc.vector.reciprocal(scale_sb, es)
        for cc in range(4):
            v = cwv[:, cc, :, oc * P:(oc + 1) * P]
            nc.vector.tensor_scalar(out=v, in0=v,
                                    scalar1=scale_sb[:, cc:cc + 1],
                                    scalar2=None, op0=Alu.mult)

    states = {}
    for bi in range(5):
        boff, bsz = BLOCKS[bi]
        esum = work.tile([P, 2, bsz], fp32, tag="esum", bufs=2,
                         name=f"esum{bi}")
        states[bi] = {"esum": esum, "pending": [],
                      "pa": [ps_pa.tile([P, bsz], fp32, tag="pa",
                                        name=f"pa{bi}_{cc}")
                             for cc in range(4)]}
        st = states[bi]
        for p0 in range(NPAIR):
            emit_pair(bi, boff, bsz, p0, st)
            if p0 == 1 and bi >= 1:
                pboff, pbsz = BLOCKS[bi - 1]
                tail_a(bi - 1, pboff, pbsz, states[bi - 1])
            while len(st["pending"]) > 2:
                emit_pa(bi, *st["pending"].pop(0), st)
            if p0 == 4 and bi >= 1:
                pboff, pbsz = BLOCKS[bi - 1]
                tail_b(bi - 1, pboff, pbsz, states[bi - 1])
            if bi == 4:
                if p0 == 6:
                    nc.vector.tensor_copy(psm[:, 0, :], pool_s)
                    nc.vector.tensor_copy(psm[:, 1, :], pool_m)
                    emit_send()
                elif p0 == 12:
                    emit_wait_combine()
                elif p0 == 14:
                    emit_mlp()
        while st["pending"]:
            emit_pa(bi, *st["pending"].pop(0), st)
    emit_sigmoid_scale(0)
    tail_a(4, *BLOCKS[4], states[4])
    tail_b(4, *BLOCKS[4], states[4])
    emit_sigmoid_scale(1)

    # ---------------- conv 3x3 + BN + ReLU -------------------------------
    for oc in range(2):
        for pt in range(4):
            y_ps = ps_pa.tile([P, 512], fp32, tag="pa")
            idx = 0
            for kh in range(3):
                for kw in range(3):
                    tnum = 3 * kh + kw
                    rs = 1 + 8 * pt + (kh - 1)
                    for ci in range(4):
                        rhs = cav4[:, ci, rs:rs + 8, kw:kw + 64]
                        nc.tensor.matmul(
                            y_ps, lhsT=cw_sb[:, tnum * 4 + ci, oc * P:(oc + 1) * P],
                            rhs=rhs, start=(idx == 0), stop=(idx == 35))
                        idx += 1
            y_sb = out_pool.tile([P, 512], fp32, tag="y_sb")
            nc.scalar.activation(y_sb, y_ps, Act.Relu,
                                 bias=bnb_sb[:, oc:oc + 1],
                                 scale=bns_sb[:, oc:oc + 1])
            nc.sync.dma_start(
                out=t["out"][oc * P:(oc + 1) * P, pt * 512:(pt + 1) * 512],
                in_=y_sb)

    ctx.close()


def build():
    if "nc" in _BUILD_CACHE:
        return _BUILD_CACHE["nc"]
    from concourse import bacc
    nc = bacc.Bacc("TRN2", target_bir_lowering=False, num_devices=8)
    f32 = mybir.dt.float32
    bf16 = mybir.dt.bfloat16
    t = {}
    t["xf"] = nc.dram_tensor("xf", [CIN, NPIX], bf16, kind="ExternalInput")
    t["xball"] = nc.dram_tensor("xball", [P, 4, M], bf16, kind="ExternalInput")
    t["pmask_bc"] = nc.dram_tensor("pmask_bc", [P, P], f32, kind="ExternalInput")
    t["wq_p"] = nc.dram_tensor("wq_p", [P, 4, C8], bf16, kind="ExternalInput")
    t["wk_p"] = nc.dram_tensor("wk_p", [P, 4, C8], bf16, kind="ExternalInput")
    t["wv_p"] = nc.dram_tensor("wv_p", [P, 4, CIN], bf16, kind="ExternalInput")
    t["bq"] = nc.dram_tensor("bq", [C8, 1], f32, kind="ExternalInput")
    t["bk2"] = nc.dram_tensor("bk2", [P, 1], f32, kind="ExternalInput")
    t["w1_p"] = nc.dram_tensor("w1_p", [P, 4, C8], bf16, kind="ExternalInput")
    t["w2_p"] = nc.dram_tensor("w2_p", [C8, 4, P], bf16, kind="ExternalInput")
    t["cw_p"] = nc.dram_tensor("cw_p", [P, 36, OC], bf16, kind="ExternalInput")
    t["bns"] = nc.dram_tensor("bns", [P, 2], f32, kind="ExternalInput")
    t["bnb"] = nc.dram_tensor("bnb", [P, 2], f32, kind="ExternalInput")
    t["out"] = nc.dram_tensor("out", [OC, OWN], f32, kind="ExternalOutput")

    with tile.TileContext(nc) as tc:
        _emit(tc, nc, t)
    nc.compile()

    _BUILD_CACHE["nc"] = nc
    return nc


def make_in_maps(x, wq, bq, wk, bk, wv, bv, ca_w1, ca_w2, conv_w,
                 bn_gamma, bn_beta, bn_mean, bn_var):
    x = np.ascontiguousarray(np.asarray(x, F32))
    B = x.shape[0]
    xf_full = x.reshape(B, CIN, NPIX)

    def part4(w):  # [CIN, K] -> [128, 4, K]
        return np.ascontiguousarray(
            np.asarray(w, F32).reshape(4, P, -1).transpose(1, 0, 2).astype(BF16))

    cw9 = np.stack([np.asarray(conv_w, F32)[:, :, kh, kw].T
                    for kh in range(3) for kw in range(3)])      # [9, CIN, OC]
    cw_p = np.ascontiguousarray(
        cw9.reshape(9, 4, P, OC).transpose(2, 0, 1, 3)
        .reshape(P, 36, OC).astype(BF16))

    common = {
        "wq_p": part4(np.asarray(wq, F32).T),
        "wk_p": part4(np.asarray(wk, F32).T),
        "wv_p": part4(np.asarray(wv, F32).T),
        "bq": np.asarray(bq, F32).reshape(C8, 1),
        "bk2": np.concatenate([np.asarray(bk, F32)] * 2).reshape(P, 1),
        "w1_p": part4(np.asarray(ca_w1, F32).T),
        "w2_p": np.ascontiguousarray(
            np.asarray(ca_w2, F32).T.reshape(C8, 4, P).astype(BF16)),
        "cw_p": cw_p,
    }
    bns = (np.asarray(bn_gamma, F32)
           / np.sqrt(np.asarray(bn_var, F32) + BN_EPS)).astype(F32)
    bnb = (np.asarray(bn_beta, F32) - np.asarray(bn_mean, F32) * bns).astype(F32)
    common["bns"] = np.ascontiguousarray(bns.reshape(2, P).T)
    common["bnb"] = np.ascontiguousarray(bnb.reshape(2, P).T)

    bv_f = np.asarray(bv, F32)
    perm = np.concatenate([np.arange(64, OWN + 64),
                           np.arange(0, 64),
                           np.arange(OWN + 64, M)])
    in_maps = []
    for core in range(8):
        b, h = core // 2, core % 2
        r0 = 32 * h - 1                       # first window row (may be -1)
        rolled = np.roll(xf_full[b], -r0 * 64, axis=1)
        xtr = rolled[:, :M] + bv_f[:, None]   # [CIN, M-window]
        if h == 0:
            xtr[:, 0:64] = 0.0
        else:
            xtr[:, M - 64:M] = 0.0
        xball = xtr[:, perm].reshape(4, P, M).transpose(1, 0, 2)
        pmask = np.ones((P, P), F32)
        if h == 0:
            pmask[:, 0:64] = 0.0
        else:
            pmask[:, 64:128] = 0.0
        in_maps.append(dict(
            common,
            xf=np.ascontiguousarray(rolled.astype(BF16)),
            xball=np.ascontiguousarray(xball.astype(BF16)),
            pmask_bc=pmask,
        ))
    return in_maps


def assemble(results):
    out = np.zeros((4, OC, 64, 64), F32)
    for core in range(8):
        b, h = core // 2, core % 2
        out[b, :, 32 * h:32 * h + 32, :] = \
            results[core]["out"].reshape(OC, 32, 64)
    return out


def kernel(**inputs):
    from concourse.bass_utils import run_bass_kernel_spmd
    nc = build()
    in_maps = make_in_maps(**inputs)
    res = run_bass_kernel_spmd(nc, in_maps, core_ids=list(range(8)))
    return assemble(res.results)


# revision 22
# speedup vs baseline: 1.0619x; 1.0619x over previous
"""DANetHead (position attention + channel attention + conv/BN/ReLU) on 8
Trainium2 NeuronCores via Bass/Tile.

Sharding: data-parallel over batch (4) x image-row-halves (2) = 8 cores.
Each core computes a 34-row window (32 own rows + 1 halo row each side) of
one batch item.  The window is position-uniform across cores (host-side roll
of the pixel axis), so one SPMD program serves all 8 cores; per-core
behaviour differs only through input data.

v2 design (vs v1):
  - attention output accumulated DIRECTLY as pa[c, m] (lhsT = v^T chunks,
    rhs = exp(e^T)) -- no PE transposes at all.
  - energy chunk-PAIRS row-packed into the 128x128 array (K=64 each half:
    even chunks on rows 0:64, odd on rows 64:128 via a partition-duplicated
    q^T and an even/odd-packed k), one Exp activation per [128, 2*bsz] pair.
  - softmax denominator via an all-ones fp32 matmul contracting the
    DVE-accumulated esum over partitions -- result lands pre-broadcast in
    all 128 partitions; reciprocal + per-column multiply on DVE.
  - residual (+bv) provided host-side in [c, m] layout (xball), fused into
    the ca write together with max/sum pooling (tensor_tensor_reduce).
  - pooled-stats pair exchange via remote_dma_broadcast (peer SBUF write,
    ~3us) instead of a CC-firmware AllGather (~27us), emitted mid-halo-block.
  - all weights host-pre-rearranged to SBUF layouts (single-descriptor DMA),
    weights-first + fine-sliced xf load order so the PE starts early.
  - conv weights SE-scaled in place, oc=0 half first so conv starts early.
"""

import numpy as np
import ml_dtypes

import os
import concourse.bass as bass
import concourse.mybir as mybir
import concourse.tile as tile

SKIP_XCHG = os.environ.get("SKIP_XCHG") == "1"
NOPACK = os.environ.get("PACK") != "1"
BF16_DEN = os.environ.get("BF16_DEN") == "1"
QDUP_PE = os.environ.get("QDUP_PE") == "1"

BF16 = ml_dtypes.bfloat16
F32 = np.float32

P = 128
CIN = 512            # channels
NPIX = 4096          # 64*64 pixels
C8 = 64              # q/k channels
OC = 256             # conv output channels
M = 2176             # per-core pixel window: 34 rows * 64
OWN = 2048           # own pixels (window rows 1..32, first in m order)
NCH = 32             # 128-pixel n-chunks
NPAIR = 16           # chunk pairs
BLOCKS = [(0, 512), (512, 512), (1024, 512), (1536, 512), (2048, 128)]

BN_EPS = 1e-5

_BUILD_CACHE = {}


def _emit(tc, nc, t):
    fp32 = mybir.dt.float32
    bf16 = mybir.dt.bfloat16
    Act = mybir.ActivationFunctionType
    Alu = mybir.AluOpType
    XY = mybir.AxisListType.XY

    import contextlib
    ctx = contextlib.ExitStack()

    persist = ctx.enter_context(tc.tile_pool(name="persist", bufs=1))
    vt_pool = ctx.enter_context(tc.tile_pool(name="vt", bufs=NCH))
    xf_pool = ctx.enter_context(tc.tile_pool(name="xf", bufs=4))
    work = ctx.enter_context(tc.tile_pool(name="work", bufs=2))
    out_pool = ctx.enter_context(tc.tile_pool(name="yout", bufs=3))
    small = ctx.enter_context(tc.tile_pool(name="small", bufs=2))

    ps_pair = ctx.enter_context(tc.tile_pool(name="ps_pair", bufs=1, space="PSUM"))
    ps_pa = ctx.enter_context(tc.tile_pool(name="ps_pa", bufs=5, space="PSUM"))
    ps_den = ctx.enter_context(tc.tile_pool(name="ps_den", bufs=1, space="PSUM"))

    # ---------------- loads (weights first, xf in fine slices) ----------
    wq_sb = persist.tile([P, 4, C8], bf16)
    nc.sync.dma_start(out=wq_sb, in_=t["wq_p"][:, :, :])
    bq_sb = persist.tile([C8, 1], fp32)
    nc.sync.dma_start(out=bq_sb, in_=t["bq"][:, :])

    xf_sb = [xf_pool.tile([P, NPIX], bf16, tag="xf", name=f"xf{ci}")
             for ci in range(4)]
    xf_groups = [(0, 512), (512, 512), (1024, 1024), (2048, 1024), (3072, 1024)]
    for s, (goff, gw) in enumerate(xf_groups):
        sl = slice(goff, goff + gw)
        if s == 2:
            wk_sb = persist.tile([P, 4, C8], bf16)
            nc.sync.dma_start(out=wk_sb, in_=t["wk_p"][:, :, :])
            bk2_sb = persist.tile([P, 1], fp32)
            nc.sync.dma_start(out=bk2_sb, in_=t["bk2"][:, :])
        if s == 3:
            wv_sb = persist.tile([P, 4, CIN], bf16)
            nc.sync.dma_start(out=wv_sb, in_=t["wv_p"][:, :, :])
        for ci in range(4):
            nc.sync.dma_start(out=xf_sb[ci][:, sl], in_=t["xf"][ci * P:(ci + 1) * P, sl])

    # late loads
    xball_sb = persist.tile([P, 4, M], bf16)
    nc.sync.dma_start(out=xball_sb, in_=t["xball"][:, :, :])
    pmask_sb = persist.tile([P, P], fp32)
    nc.sync.dma_start(out=pmask_sb, in_=t["pmask_bc"][:, :])
    w1_sb = persist.tile([P, 4, C8], bf16)
    nc.sync.dma_start(out=w1_sb, in_=t["w1_p"][:, :, :])
    w2_sb = persist.tile([C8, 4, P], bf16)
    nc.sync.dma_start(out=w2_sb, in_=t["w2_p"][:, :, :])
    cw_sb = persist.tile([P, 36, OC], bf16)
    nc.sync.dma_start(out=cw_sb, in_=t["cw_p"][:, :, :])
    bns_sb = persist.tile([P, 2], fp32)
    nc.sync.dma_start(out=bns_sb, in_=t["bns"][:, :])
    bnb_sb = persist.tile([P, 2], fp32)
    nc.sync.dma_start(out=bnb_sb, in_=t["bnb"][:, :])

    # Dummy 8-core AllGather: its only purpose is to make the NEFF a CC
    # participant so the runtime co-schedules all 8 cores (otherwise cores
    # launch ms apart and the P2P stats exchange eats the skew).  Runs on
    # the CC cores during the load phase; nothing reads its output.
    rsem = nc.alloc_semaphore("xchg_r")
    lsem = nc.alloc_semaphore("xchg_l")
    psem = nc.alloc_semaphore("xchg_p")
    dram = ctx.enter_context(tc.tile_pool(name="dram", bufs=1, space="DRAM"))
    bar_sb = small.tile([P, 1], fp32, tag="bar", bufs=1)
    nc.vector.memset(bar_sb, 0.0)
    bar_in = dram.tile([P, 1], fp32, tag="bar_in")
    bar_out = dram.tile([8, P, 1], fp32, tag="bar_out")
    nc.sync.dma_start(out=bar_in, in_=bar_sb)
    nc.gpsimd.collective_compute(
        "AllGather", Alu.bypass,
        replica_groups=[[0, 1, 2, 3, 4, 5, 6, 7]],
        ins=[bar_in.opt()], outs=[bar_out.opt()])
    with tc.tile_critical(name="semclear", no_gpsimd_drain=True):
        nc.gpsimd.sem_clear(rsem)
        nc.gpsimd.sem_clear(lsem)
        nc.gpsimd.sem_clear(psem)

    ones_sb = persist.tile([P, P], fp32)
    nc.vector.memset(ones_sb, 1.0)
    ones16_sb = persist.tile([P, P], bf16)
    nc.vector.memset(ones16_sb, 1.0)

    # ca: [c-part, cc, 34 rows, 66 cols], zero column pads
    ca_sb = persist.tile([P, 4, 34 * 66], bf16)
    cav4 = ca_sb.rearrange("p c (r x) -> p c r x", x=66)
    for cc in range(4):
        nc.vector.memset(cav4[:, cc, :, 0:1], 0.0)
        nc.vector.memset(cav4[:, cc, :, 65:66], 0.0)

    # ---------------- q projection -> qT2 (duplicated partitions) -------
    qT2 = persist.tile([P, M], bf16)
    for b in range(4):
        q_ps = ps_den.tile([C8, 512], fp32, tag="den")
        for ci in range(4):
            nc.tensor.matmul(q_ps, lhsT=wq_sb[:, ci, :],
                             rhs=xf_sb[ci][:, 64 + 512 * b:576 + 512 * b],
                             start=(ci == 0), stop=(ci == 3))
        nc.scalar.activation(qT2[0:C8, 512 * b:512 * (b + 1)], q_ps,
                             Act.Identity, bias=bq_sb[:, 0:1])
    qh_ps = ps_den.tile([C8, P], fp32, tag="den")
    for ci in range(4):
        nc.tensor.matmul(qh_ps[:, 0:64], lhsT=wq_sb[:, ci, :],
                         rhs=xf_sb[ci][:, 0:64], start=(ci == 0), stop=(ci == 3))
    for ci in range(4):
        nc.tensor.matmul(qh_ps[:, 64:128], lhsT=wq_sb[:, ci, :],
                         rhs=xf_sb[ci][:, OWN + 64:OWN + 128],
                         start=(ci == 0), stop=(ci == 3))
    nc.scalar.activation(qT2[0:C8, OWN:OWN + P], qh_ps,
                         Act.Identity, bias=bq_sb[:, 0:1])
    # duplicate q^T to partitions 64:128
    if QDUP_PE:
        for b in range(4):
            q_ps2 = ps_den.tile([C8, 512], fp32, tag="den", name=f"qps2_{b}")
            for ci in range(4):
                nc.tensor.matmul(q_ps2, lhsT=wq_sb[:, ci, :],
                                 rhs=xf_sb[ci][:, 64 + 512 * b:576 + 512 * b],
                                 start=(ci == 0), stop=(ci == 3))
            nc.scalar.activation(qT2[C8:P, 512 * b:512 * (b + 1)], q_ps2,
                                 Act.Identity, bias=bq_sb[:, 0:1])
        qh_ps2 = ps_den.tile([C8, P], fp32, tag="den")
        for ci in range(4):
            nc.tensor.matmul(qh_ps2[:, 0:64], lhsT=wq_sb[:, ci, :],
                             rhs=xf_sb[ci][:, 0:64], start=(ci == 0), stop=(ci == 3))
        for ci in range(4):
            nc.tensor.matmul(qh_ps2[:, 64:128], lhsT=wq_sb[:, ci, :],
                             rhs=xf_sb[ci][:, OWN + 64:OWN + 128],
                             start=(ci == 0), stop=(ci == 3))
        nc.scalar.activation(qT2[C8:P, OWN:OWN + P], qh_ps2,
                             Act.Identity, bias=bq_sb[:, 0:1])
    else:
        nc.sync.dma_start(out=qT2[C8:P, :], in_=qT2[0:C8, :])

    # ---------------- k projection, even/odd packed ----------------------
    # kpack[0:64, p0, :] = k for chunk 2*p0 ; kpack[64:128, p0, :] = 2*p0+1
    if NOPACK:
        k_sb = persist.tile([C8, NPIX], bf16)
        for j in range(8):
            k_ps = ps_den.tile([C8, 512], fp32, tag="den", name=f"kps{j}")
            for ci in range(4):
                nc.tensor.matmul(k_ps, lhsT=wk_sb[:, ci, :],
                                 rhs=xf_sb[ci][:, 512 * j:512 * (j + 1)],
                                 start=(ci == 0), stop=(ci == 3))
            nc.scalar.activation(k_sb[:, 512 * j:512 * (j + 1)], k_ps,
                                 Act.Identity, bias=bq_sb[:, 0:1] if False else bk2_sb[0:C8, 0:1])
        kpack = None
    else:
        kpack = persist.tile([P, NPAIR, P], bf16)
        xv = [xf_sb[ci].rearrange("p (a e b) -> p a e b", e=2, b=P)
              for ci in range(4)]   # a = pair index, e = even/odd within pair
        for j in range(8):          # 512-pixel column blocks (pairs 2j, 2j+1)
            kp_ps = ps_den.tile([P, 2, P], fp32, tag="den", name=f"kpps{j}")
            for ci in range(4):     # even chunks -> partitions 0:64
                nc.tensor.matmul(kp_ps[0:C8, :, :], lhsT=wk_sb[:, ci, :],
                                 rhs=xv[ci][:, 2 * j:2 * j + 2, 0, :],
                                 start=(ci == 0), stop=(ci == 3))
            for ci in range(4):     # odd chunks -> partitions 64:128
                nc.tensor.matmul(kp_ps[C8:P, :, :], lhsT=wk_sb[:, ci, :],
                                 rhs=xv[ci][:, 2 * j:2 * j + 2, 1, :],
                                 start=(ci == 0), stop=(ci == 3))
            nc.scalar.activation(kpack[:, 2 * j:2 * j + 2, :], kp_ps,
                                 Act.Identity, bias=bk2_sb[:, 0:1])

    # ---------------- v^T ------------------------------------------------
    vt_sb = []
    for nch in range(NCH):
        v_ps = ps_pa.tile([P, CIN], fp32, tag="pa")
        for ci in range(4):
            nc.tensor.matmul(v_ps, lhsT=xf_sb[ci][:, nch * P:(nch + 1) * P],
                             rhs=wv_sb[:, ci, :], start=(ci == 0), stop=(ci == 3))
        vt = vt_pool.tile([P, CIN], bf16, tag="vt")
        nc.vector.tensor_copy(vt, v_ps)
        vt_sb.append(vt)

    # ---------------- attention -----------------------------------------
    pool_s = small.tile([P, 4], fp32, tag="pool_s", bufs=1)
    pool_m = small.tile([P, 4], fp32, tag="pool_m", bufs=1)
    pool_sm = persist.tile([P, 2 * 4], fp32)     # [ (2 stats, 4 cc) ]
    pool_rm = persist.tile([P, 2 * 4], fp32)     # partner's, remote-written

    def emit_pair(bi, boff, bsz, p0, state):
        """energy pair + exp + esum; 1-lag pa of previous pair."""
        pair_ps = ps_pair.tile([P, 2, bsz], fp32, tag="pair", name=f"pr{bi}_{p0}")
        if NOPACK:
            nc.tensor.matmul(pair_ps[:, 0, :],
                             lhsT=k_sb[:, (2 * p0) * P:(2 * p0 + 1) * P],
                             rhs=qT2[0:C8, boff:boff + bsz], start=True, stop=True)
            nc.tensor.matmul(pair_ps[:, 1, :],
                             lhsT=k_sb[:, (2 * p0 + 1) * P:(2 * p0 + 2) * P],
                             rhs=qT2[0:C8, boff:boff + bsz], start=True, stop=True)
        else:
            nc.tensor.matmul(pair_ps[:, 0, :], lhsT=kpack[0:C8, p0, :],
                             rhs=qT2[0:C8, boff:boff + bsz], start=True, stop=True)
            nc.tensor.matmul(pair_ps[:, 1, :], lhsT=kpack[C8:P, p0, :],
                             rhs=qT2[C8:P, boff:boff + bsz], start=True, stop=True)
        expt = work.tile([P, 2, bsz], bf16, tag="expt", bufs=3)
        nc.scalar.activation(expt, pair_ps, Act.Exp)
        if p0 == 0:
            nc.vector.tensor_copy(state["esum"], expt)
        else:
            nc.vector.tensor_add(state["esum"], state["esum"], expt)
        state["pending"].append((p0, expt))

    def emit_pa(bi, p0, expt, state):
        for h in range(2):
            nch = 2 * p0 + h
            for cc in range(4):
                nc.tensor.matmul(state["pa"][cc],
                                 lhsT=vt_sb[nch][:, cc * P:(cc + 1) * P],
                                 rhs=expt[:, h, :],
                                 start=(nch == 0), stop=(nch == NCH - 1))

    def tail_a(bi, boff, bsz, state):
        """den (pre-broadcast), recip, pnorm -- the pa-bank readers."""
        den_ps = ps_den.tile([P, bsz], fp32, tag="den", name=f"den{bi}")
        if BF16_DEN:
            esb = work.tile([P, 2, bsz], bf16, tag="esb", bufs=2,
                            name=f"esb{bi}")
            nc.vector.tensor_copy(esb, state["esum"])
            nc.tensor.matmul(den_ps, lhsT=ones16_sb, rhs=esb[:, 0, :],
                             start=True, stop=False)
            nc.tensor.matmul(den_ps, lhsT=ones16_sb, rhs=esb[:, 1, :],
                             start=False, stop=True)
        else:
            nc.tensor.matmul(den_ps, lhsT=ones_sb, rhs=state["esum"][:, 0, :],
                             start=True, stop=False)
            nc.tensor.matmul(den_ps, lhsT=ones_sb, rhs=state["esum"][:, 1, :],
                             start=False, stop=True)
        recip = work.tile([P, bsz], fp32, tag="recip", bufs=2,
                          name=f"recip{bi}")
        nc.vector.reciprocal_approx_fast(out=recip, in_=den_ps)
        if bi == 4:
            nc.vector.tensor_mul(recip, recip, pmask_sb)
        state["pnorm"] = []
        for cc in range(4):
            pnorm = work.tile([P, bsz], fp32, tag="pnorm", bufs=4,
                              name=f"pn{bi}_{cc}")
            nc.vector.tensor_mul(pnorm, state["pa"][cc], recip)
            state["pnorm"].append(pnorm)

    def tail_b(bi, boff, bsz, state):
        """normalized + residual -> ca (+ pooling for own blocks)."""
        r0 = 1 + 8 * bi
        for cc in range(4):
            pnorm = state["pnorm"][cc]
            xb = xball_sb[:, cc, boff:boff + bsz]
            if bi < 4:
                cav = cav4[:, cc, r0:r0 + 8, 1:65]
                pn3 = pnorm.rearrange("p (r x) -> p r x", x=64)
                xb3 = xb.rearrange("p (r x) -> p r x", x=64)
                ptm = small.tile([P, 4, 2], fp32, tag="ptm")
                nc.vector.tensor_add(cav, pn3, xb3)
                nc.vector.reduce_max(ptm[:, cc, 0:1], cav, axis=XY)
                nc.vector.reduce_sum(ptm[:, cc, 1:2], cav, axis=XY)
                if bi == 0:
                    nc.vector.tensor_copy(pool_m[:, cc:cc + 1], ptm[:, cc, 0:1])
                    nc.vector.tensor_copy(pool_s[:, cc:cc + 1], ptm[:, cc, 1:2])
                else:
                    nc.vector.tensor_max(pool_m[:, cc:cc + 1],
                                         pool_m[:, cc:cc + 1], ptm[:, cc, 0:1])
                    nc.vector.tensor_add(pool_s[:, cc:cc + 1],
                                         pool_s[:, cc:cc + 1], ptm[:, cc, 1:2])
            else:
                nc.vector.tensor_add(cav4[:, cc, 0:1, 1:65],
                                     pnorm[:, 0:64], xb[:, 0:64])
                nc.vector.tensor_add(cav4[:, cc, 33:34, 1:65],
                                     pnorm[:, 64:128], xb[:, 64:128])

    # ---- attention blocks, software-pipelined tails ----
    # block bi's tail (den/recip/normalize/ca) is emitted 3 pairs into block
    # bi+1 so the PE never waits on the DVE tail chain.  The SE-stats
    # exchange + MLP are interleaved with the halo block's pairs.
    psm = pool_sm.rearrange("p (s c) -> p s c", c=4)
    prm = pool_rm.rearrange("p (s c) -> p s c", c=4)
    zs_sb = small.tile([P, 4], fp32, tag="zs")
    zm_sb = small.tile([P, 4], fp32, tag="zm")
    rhs_z = small.tile([P, 4, 2], bf16, tag="rhs_z")
    h_sb = small.tile([C8, 2], bf16, tag="h_sb")
    stot = small.tile([P, 4], fp32, tag="stot")
    s_sb = small.tile([P, 4, 2], fp32, tag="s_sb", bufs=1)
    es = small.tile([P, 4], fp32, tag="es")
    scale_sb = small.tile([P, 4], fp32, tag="scale")
    cwv = cw_sb.rearrange("p (t c) o -> p c t o", c=4)

    def emit_send():
        if SKIP_XCHG:
            return
        with tc.tile_critical(name="xchg_send", no_gpsimd_drain=True):
            nc.gpsimd.remote_dma_broadcast(
                out_ap=pool_rm[:, :], in_ap=pool_sm[:, :],
                remote_sem=rsem, local_sem=lsem,
                rdests=[(0, 1), None, None, None, None, None, None, None],
            ).then_inc(psem, 1)
            nc.gpsimd.wait_ge(psem, 1)
            nc.gpsimd.trigger_dma(count=1)

    def emit_wait_combine():
        if SKIP_XCHG:
            nc.vector.tensor_copy(pool_rm[:, :], pool_sm[:, :])
        else:
            with tc.tile_critical(name="xchg_wait"):
                nc.vector.wait_ge(rsem, 2)
        nc.vector.tensor_add(zs_sb, psm[:, 0, :], prm[:, 0, :])
        nc.vector.tensor_max(zm_sb, psm[:, 1, :], prm[:, 1, :])
        nc.vector.tensor_scalar_mul(rhs_z[:, :, 0], zs_sb, 1.0 / float(NPIX))
        nc.vector.tensor_copy(rhs_z[:, :, 1], zm_sb)

    def emit_mlp():
        h_ps = ps_den.tile([C8, 2], fp32, tag="den")
        for cc in range(4):
            nc.tensor.matmul(h_ps, lhsT=w1_sb[:, cc, :], rhs=rhs_z[:, cc, :],
                             start=(cc == 0), stop=(cc == 3))
        nc.vector.tensor_scalar_max(h_sb, h_ps, 0.0)
        for cc in range(4):
            s_ps = ps_den.tile([P, 2], fp32, tag="den", name=f"s_ps{cc}")
            nc.tensor.matmul(s_ps, lhsT=w2_sb[:, cc, :], rhs=h_sb,
                             start=True, stop=True)
            nc.vector.tensor_copy(s_sb[:, cc, :], s_ps)
            nc.vector.tensor_add(stot[:, cc:cc + 1], s_sb[:, cc, 0:1],
                                 s_sb[:, cc, 1:2])

    def emit_sigmoid_scale(oc):
        if oc == 0:
            nc.scalar.activation(es, stot, Act.Exp, scale=-1.0)
            nc.vector.tensor_scalar_add(es, es, 1.0)
            nc.vector.reciprocal(scale_sb, es)
        for cc in range(4):
            v = cwv[:, cc, :, oc * P:(oc + 1) * P]
            nc.vector.tensor_scalar(out=v, in0=v,
                                    scalar1=scale_sb[:, cc:cc + 1],
                                    scalar2=None, op0=Alu.mult)

    states = {}
    for bi in range(5):
        boff, bsz = BLOCKS[bi]
        esum = work.tile([P, 2, bsz], fp32, tag="esum", bufs=2,
                         name=f"esum{bi}")
        states[bi] = {"esum": esum, "pending": [],
                      "pa": [ps_pa.tile([P, bsz], fp32, tag="pa",
                                        name=f"pa{bi}_{cc}")
                             for cc in range(4)]}
        st = states[bi]
        for p0 in range(NPAIR):
            emit_pair(bi, boff, bsz, p0, st)
            if p0 == 1 and bi >= 1:
                pboff, pbsz = BLOCKS[bi - 1]
                tail_a(bi - 1, pboff, pbsz, states[bi - 1])
            while len(st["pending"]) > 2:
                emit_pa(bi, *st["pending"].pop(0), st)
            if p0 == 4 and bi >= 1:
                pboff, pbsz = BLOCKS[bi - 1]
                tail_b(bi - 1, pboff, pbsz, states[bi - 1])
            if bi == 4:
                if p0 == 6:
                    nc.vector.tensor_copy(psm[:, 0, :], pool_s)
                    nc.vector.tensor_copy(psm[:, 1, :], pool_m)
                    emit_send()
                elif p0 == 12:
                    emit_wait_combine()
                elif p0 == 14:
                    emit_mlp()
        while st["pending"]:
            emit_pa(bi, *st["pending"].pop(0), st)
    emit_sigmoid_scale(0)
    tail_a(4, *BLOCKS[4], states[4])
    tail_b(4, *BLOCKS[4], states[4])
    emit_sigmoid_scale(1)

    # ---------------- conv 3x3 + BN + ReLU -------------------------------
    for oc in range(2):
        for pt in range(4):
            y_ps = ps_pa.tile([P, 512], fp32, tag="pa")
            idx = 0
            for kh in range(3):
                for kw in range(3):
                    tnum = 3 * kh + kw
                    rs = 1 + 8 * pt + (kh - 1)
                    for ci in range(4):
                        rhs = cav4[:, ci, rs:rs + 8, kw:kw + 64]
                        nc.tensor.matmul(
                            y_ps, lhsT=cw_sb[:, tnum * 4 + ci, oc * P:(oc + 1) * P],
                            rhs=rhs, start=(idx == 0), stop=(idx == 35))
                        idx += 1
            y_sb = out_pool.tile([P, 512], fp32, tag="y_sb")
            nc.scalar.activation(y_sb, y_ps, Act.Relu,
                                 bias=bnb_sb[:, oc:oc + 1],
                                 scale=bns_sb[:, oc:oc + 1])
            nc.sync.dma_start(
                out=t["out"][oc * P:(oc + 1) * P, pt * 512:(pt + 1) * 512],
                in_=y_sb)

    ctx.close()


def build():
    if "nc" in _BUILD_CACHE:
        return _BUILD_CACHE["nc"]
    from concourse import bacc
    nc = bacc.Bacc("TRN2", target_bir_lowering=False, num_devices=8)
    f32 = mybir.dt.float32
    bf16 = mybir.dt.bfloat16
    t = {}
    t["xf"] = nc.dram_tensor("xf", [CIN, NPIX], bf16, kind="ExternalInput")
    t["xball"] = nc.dram_tensor("xball", [P, 4, M], bf16, kind="ExternalInput")
    t["pmask_bc"] = nc.dram_tensor("pmask_bc", [P, P], f32, kind="ExternalInput")
    t["wq_p"] = nc.dram_tensor("wq_p", [P, 4, C8], bf16, kind="ExternalInput")
    t["wk_p"] = nc.dram_tensor("wk_p", [P, 4, C8], bf16, kind="ExternalInput")
    t["wv_p"] = nc.dram_tensor("wv_p", [P, 4, CIN], bf16, kind="ExternalInput")
    t["bq"] = nc.dram_tensor("bq", [C8, 1], f32, kind="ExternalInput")
    t["bk2"] = nc.dram_tensor("bk2", [P, 1], f32, kind="ExternalInput")
    t["w1_p"] = nc.dram_tensor("w1_p", [P, 4, C8], bf16, kind="ExternalInput")
    t["w2_p"] = nc.dram_tensor("w2_p", [C8, 4, P], bf16, kind="ExternalInput")
    t["cw_p"] = nc.dram_tensor("cw_p", [P, 36, OC], bf16, kind="ExternalInput")
    t["bns"] = nc.dram_tensor("bns", [P, 2], f32, kind="ExternalInput")
    t["bnb"] = nc.dram_tensor("bnb", [P, 2], f32, kind="ExternalInput")
    t["out"] = nc.dram_tensor("out", [OC, OWN], f32, kind="ExternalOutput")

    with tile.TileContext(nc) as tc:
        _emit(tc, nc, t)
    nc.compile()

    _BUILD_CACHE["nc"] = nc
    return nc


def make_in_maps(x, wq, bq, wk, bk, wv, bv, ca_w1, ca_w2, conv_w,
                 bn_gamma, bn_beta, bn_mean, bn_var):
    x = np.ascontiguousarray(np.asarray(x, F32))
    B = x.shape[0]
    xf_full = x.reshape(B, CIN, NPIX)

    def part4(w):  # [CIN, K] -> [128, 4, K]
        return np.ascontiguousarray(
            np.asarray(w, F32).reshape(4, P, -1).transpose(1, 0, 2).astype(BF16))

    cw9 = np.stack([np.asarray(conv_w, F32)[:, :, kh, kw].T
                    for kh in range(3) for kw in range(3)])      # [9, CIN, OC]
    cw_p = np.ascontiguousarray(
        cw9.reshape(9, 4, P, OC).transpose(2, 0, 1, 3)
        .reshape(P, 36, OC).astype(BF16))

    common = {
        "wq_p": part4(np.asarray(wq, F32).T),
        "wk_p": part4(np.asarray(wk, F32).T),
        "wv_p": part4(np.asarray(wv, F32).T),
        "bq": np.asarray(bq, F32).reshape(C8, 1),
        "bk2": np.concatenate([np.asarray(bk, F32)] * 2).reshape(P, 1),
        "w1_p": part4(np.asarray(ca_w1, F32).T),
        "w2_p": np.ascontiguousarray(
            np.asarray(ca_w2, F32).T.reshape(C8, 4, P).astype(BF16)),
        "cw_p": cw_p,
    }
    bns = (np.asarray(bn_gamma, F32)
           / np.sqrt(np.asarray(bn_var, F32) + BN_EPS)).astype(F32)
    bnb = (np.asarray(bn_beta, F32) - np.asarray(bn_mean, F32) * bns).astype(F32)
    common["bns"] = np.ascontiguousarray(bns.reshape(2, P).T)
    common["bnb"] = np.ascontiguousarray(bnb.reshape(2, P).T)

    bv_f = np.asarray(bv, F32)
    perm = np.concatenate([np.arange(64, OWN + 64),
                           np.arange(0, 64),
                           np.arange(OWN + 64, M)])
    in_maps = []
    for core in range(8):
        b, h = core // 2, core % 2
        r0 = 32 * h - 1                       # first window row (may be -1)
        rolled = np.roll(xf_full[b], -r0 * 64, axis=1)
        xtr = rolled[:, :M] + bv_f[:, None]   # [CIN, M-window]
        if h == 0:
            xtr[:, 0:64] = 0.0
        else:
            xtr[:, M - 64:M] = 0.0
        xball = xtr[:, perm].reshape(4, P, M).transpose(1, 0, 2)
        pmask = np.ones((P, P), F32)
        if h == 0:
            pmask[:, 0:64] = 0.0
        else:
            pmask[:, 64:128] = 0.0
        in_maps.append(dict(
            common,
            xf=np.ascontiguousarray(rolled.astype(BF16)),
            xball=np.ascontiguousarray(xball.astype(BF16)),
            pmask_bc=pmask,
        ))
    return in_maps


def assemble(results):
    out = np.zeros((4, OC, 64, 64), F32)
    for core in range(8):
        b, h = core // 2, core % 2
        out[b, :, 32 * h:32 * h + 32, :] = \
            results[core]["out"].reshape(OC, 32, 64)
    return out


def kernel(**inputs):
    from concourse.bass_utils import run_bass_kernel_spmd
    nc = build()
    in_maps = make_in_maps(**inputs)
    res = run_bass_kernel_spmd(nc, in_maps, core_ids=list(range(8)))
    return assemble(res.results)
